# revision 42
# baseline (speedup 1.0000x reference)
"""GroupedQueryAttention, tensor-parallel over heads on 8 NeuronCores (raw Bass).

Core c owns q heads {2c, 2c+1} and kv head c//2. All matmul operands bf16
(f32 PSUM), inputs host-cast/transposed/pre-arranged for contiguous DMA.
Device pipeline per core:
  load hsT (host-transposed, chunk-contiguous) -> qkv projections (PE,
  q 2-head-packed, k duplicated into both partition halves) -> RoPE (DVE;
  swap-halves DMAs: q on ACT queue, k on gpsimd queue; sign folded into
  host ssin table) ->
  per head h: S^T[k,q] = kT2[h*64:+64].T @ qT2[h*64:+64] (PE quadrant) ->
  P^T = exp(0.125 S^T) (ACT, bf16 out, ring of 16) ->
  pv = [V|1].T @ P^T (PE, fused denominator row) ->
  raw-copy pv -> SBUF (releases PSUM to next head fast) ->
  den row -> [64,32] via DMA, recip (DVE), scr2 DRAM, stride-0 broadcast,
  normalize muls (DVE) -> AllGather per head (h0's hides under h1 attn) ->
  o_proj slab0 pass during h1's collective, slab1 accumulate after.
Host: out[:, c*128:(c+1)*128] = out_t_c.T.
"""
import sys, os
sys.path.insert(0, '/opt/trn_rl_repo')
import contextlib
import numpy as np
import ml_dtypes
import concourse.bass as bass
import concourse.mybir as mybir
from concourse.bass_utils import run_bass_kernel_spmd

F32 = mybir.dt.float32
BF16 = mybir.dt.bfloat16
EXP = mybir.ActivationFunctionType.Exp
NPBF = ml_dtypes.bfloat16

S, HID, HD = 2048, 1024, 64
NCORES = 8
NST = S // 128      # 16 k tiles
NHT = HID // 128    # 8 contraction tiles
NSC = 4             # 512-wide seq chunks
NPT = 16            # PT ring size (must divide 2*NST)


def build_kernel():
    nc = bass.Bass("TRN2", target_bir_lowering=False, num_devices=NCORES)

    hsT_d = nc.dram_tensor("hst", [NSC, 128, NHT * 512], BF16, kind="ExternalInput")
    wq_d = nc.dram_tensor("wq", [128, NHT * 128], BF16, kind="ExternalInput")
    wkk_d = nc.dram_tensor("wkk", [128, NHT * 128], BF16, kind="ExternalInput")
    wv_d = nc.dram_tensor("wv", [128, NHT * HD], BF16, kind="ExternalInput")
    wo_d = nc.dram_tensor("wo", [128, NHT * 128], BF16, kind="ExternalInput")
    cosT_d = nc.dram_tensor("cost", [128, S], BF16, kind="ExternalInput")
    ssinT_d = nc.dram_tensor("ssint", [128, S], BF16, kind="ExternalInput")
    id_d = nc.dram_tensor("ident", [128, 128], BF16, kind="ExternalInput")
    out_d = nc.dram_tensor("out_t", [128, S], F32, kind="ExternalOutput")
    scr2_d = nc.dram_tensor("scr2", [2, S], F32)
    agin_d = nc.dram_tensor("agin", [2, HD, S], BF16)
    agout_d = nc.dram_tensor("agout", [2, NCORES, HD, S], BF16, addr_space="Shared")
    bar_d = nc.dram_tensor("bar", [16384], F32)
    barout_d = nc.dram_tensor("barout", [NCORES, 16384], F32, addr_space="Shared")

    def sb(name, shape, dt):
        return nc.alloc_sbuf_tensor(name, shape, dt).ap()

    hsT = sb("hsT", [128, NHT, S], BF16)
    ident = sb("ident_sb", [128, 128], BF16)
    cosT2 = sb("cosT2", [128, S], BF16)
    ssinT2 = sb("ssinT2", [128, S], BF16)
    wq_sb = sb("wq_sb", [128, NHT, 128], BF16)
    wkk_sb = sb("wkk_sb", [128, NHT, 128], BF16)
    wv_sb = sb("wv_sb", [128, NHT, HD], BF16)
    wo_sb = sb("wo_sb", [128, NHT, 128], BF16)
    q2 = [sb(f"q2_{i}", [128, 512], BF16) for i in range(2)]
    qs2 = [sb(f"qs2_{i}", [128, 512], BF16) for i in range(2)]
    k2 = [sb(f"k2_{i}", [128, 512], BF16) for i in range(2)]
    ks2 = [sb(f"ks2_{i}", [128, 512], BF16) for i in range(2)]
    tmpa = sb("tmpa", [128, 512], BF16)
    tmpb = sb("tmpb", [128, 512], BF16)
    qT2 = sb("qT2", [128, S], BF16)
    kT2 = sb("kT2", [128, S], BF16)
    vT = sb("vT", [HD, S], BF16)
    vaug = sb("vaug", [128, NST, HD + 1], BF16)
    PT = [sb(f"PT{i}", [128, 1024], BF16) for i in range(NPT)]
    araw = [sb(f"araw{h}", [HD + 1, S], BF16) for h in range(2)]
    den64 = sb("den64", [64, 32], BF16)
    rcp64 = sb("rcp64", [64, 32], F32)
    rb = [sb(f"rb{i}", [HD, 512], F32) for i in range(4)]
    attn_sb = [sb(f"attn{h}", [HD, S], BF16) for h in range(2)]
    af = [sb(f"af{h}", [128, 4, S], BF16) for h in range(2)]
    out_ch = [sb(f"out_ch{i}", [128, 512], F32) for i in range(2)]
    dum_sb = sb("dum_sb", [1, 16], F32)
    dumb_sb = sb("dumb_sb", [1, 16], BF16)

    ps = nc.alloc_psum_tensor("psblob", [128, 4096], F32).ap()
    qp = [ps[:, 0:512], ps[:, 512:1024]]
    kp = [ps[:, 1024:1536], ps[:, 1536:2048]]
    vp = [ps[0:HD, 2048:2560], ps[0:HD, 2560:3072]]
    vtp = ps[:, 3072:3584].bitcast(BF16)          # [128, 1024]: 16 x [128,64]
    sp = [ps[:, 0:1024], ps[:, 1024:2048]]
    pv = ps[0:HD + 1, 2048:4096]                  # [65, 2048]
    op4 = [ps[:, oc * 512:(oc + 1) * 512] for oc in range(NSC)]

    es = contextlib.ExitStack()
    SEM = lambda n: es.enter_context(nc.semaphore(n))
    sL = SEM("sL")        # SP weight/table loads (+16)
    sHSa = SEM("sHSa")    # hsT chunks 0,2 (SP)
    sHSb = SEM("sHSb")    # hsT chunks 1,3 (gpsimd)
    sQP = SEM("sQP"); sKP = SEM("sKP"); sVP = SEM("sVP")
    sQC = SEM("sQC"); sKC = SEM("sKC"); sVC = SEM("sVC")
    sSWQ = SEM("sSWQ")    # q swap DMAs (+16 each, 4/chunk, ACT queue)
    sSWK = SEM("sSWK")    # k swap DMAs (+16 each, 4/chunk, gpsimd queue)
    sQR = SEM("sQR"); sKR = SEM("sKR")
    sVA = SEM("sVA"); sMS = SEM("sMS")
    sSC = SEM("sSC"); sEX = SEM("sEX"); sPV = SEM("sPV")
    sPVD = SEM("sPVD")    # PE drain after each head's final PV (1/h)
    sVTD = SEM("sVTD")    # PE drain after each chunk's v transposes
    sRW = SEM("sRW")      # raw attn copies out of psum (1 per (h,oc))
    sDN64 = SEM("sDN64")  # den64 DMAs (+16/h)
    sRC = SEM("sRC")      # recips (1/h)
    sDNS = SEM("sDNS")    # scr2 writes (+16/h)
    sRB = SEM("sRB")      # rb broadcasts (+16 per (h,oc))
    sNM = SEM("sNM")      # normalize muls (1 per (h,oc))
    sAG = SEM("sAG")      # agin DMAs (+16/h)
    sCC = SEM("sCC")      # collectives (1/h)
    sAF = SEM("sAF")      # af loads, all on SP queue (+16 each)
    sOP = SEM("sOP")      # o_proj slab1 stop (1/oc)
    sOC = SEM("sOC"); sOD = SEM("sOD")

    def _pv(tensor, h, kt):
        tensor.wait_ge(sEX, h * 32 + 2 * kt + 2)
        if h == 1 and kt == 0:
            tensor.wait_ge(sRW, 4)  # head0 raw copies done, pv psum free
        for qc in range(4):
            slot = (2 * kt + qc // 2) % NPT
            inst = tensor.matmul(
                pv[:, qc * 512:(qc + 1) * 512],
                vaug[:, kt, :],
                PT[slot][:, (qc % 2) * 512:(qc % 2 + 1) * 512],
                start=(kt == 0), stop=(kt == NST - 1),
            )
        inst.then_inc(sPV, 1)

    with nc.Block() as block:

        # ===== SP: loads + den chain + collectives + af + stores =====
        @block.sync
        def _(sync):
            sync.dma_start(out=wkk_sb, in_=wkk_d[:]).then_inc(sL, 16)
            sync.dma_start(out=hsT[:, :, 0:512], in_=hsT_d[0]).then_inc(sHSa, 16)
            sync.dma_start(out=wq_sb, in_=wq_d[:]).then_inc(sL, 16)
            sync.dma_start(out=wv_sb, in_=wv_d[:]).then_inc(sL, 16)
            sync.dma_start(out=hsT[:, :, 1024:1536], in_=hsT_d[2]).then_inc(sHSa, 16)
            sync.dma_start(out=ident, in_=id_d[:]).then_inc(sL, 16)
            sync.dma_start(out=cosT2, in_=cosT_d[:]).then_inc(sL, 16)
            sync.dma_start(out=ssinT2, in_=ssinT_d[:]).then_inc(sL, 16)
            sync.dma_start(out=wo_sb, in_=wo_d[:]).then_inc(sL, 16)
            # den chain per head: plain transfers on SP hardware DGE; the
            # reshaping/broadcast DMAs live on gpsimd (software DGE)
            for h in range(2):
                sync.wait_ge(sRC, h + 1)
                sync.dma_start(out=scr2_d[h], in_=rcp64).then_inc(sDNS, 16)
                # read-back: its completion proves the write is in DRAM and
                # visible to the gpsimd queue's broadcast reads
                sync.dma_start(out=dum_sb, in_=scr2_d[h, 0:16]).then_inc(sDNS, 16)
                sync.wait_ge(sNM, 4 * (h + 1))
                sync.dma_start(out=agin_d[h], in_=attn_sb[h]).then_inc(sAG, 16)
                sync.dma_start(out=dumb_sb, in_=agin_d[h][0:1, 0:16]).then_inc(sAG, 16)
            # A collective's completion sem does NOT order remote ranks'
            # inbound pushes against our reads. Per-sender SDMA queues are
            # FIFO, so entering the NEXT collective proves every rank's
            # previous pushes landed: gate af[0] on cc1, af[1] on the
            # trailing barrier collective.
            for h in range(2):
                sync.wait_ge(sCC, 2 + h)
                for oc in range(NSC):
                    sync.dma_start(
                        out=af[h][:, :, oc * 512:(oc + 1) * 512],
                        in_=agout_d[h].rearrange(
                            "(t a) d (n q) -> (a d) t n q", a=2, q=512
                        )[:, :, oc, :],
                    ).then_inc(sAF, 16)
            sync.dma_start(out=dum_sb, in_=scr2_d[0, 0:16]).then_inc(sAF, 16)
            for oc in range(NSC):
                sync.wait_ge(sOC, oc + 1)
                sync.dma_start(out=out_d[:, oc * 512:(oc + 1) * 512], in_=out_ch[oc % 2]).then_inc(sOD, 16)
            sync.wait_ge(sOD, 16 * NSC)

        # ================= PE =================
        @block.tensor
        def _(tensor):
            for sc in range(NSC):
                if sc % 2 == 0:
                    tensor.wait_ge(sHSa, 16 * (sc // 2 + 1))
                else:
                    tensor.wait_ge(sHSb, 16 * (sc // 2 + 1))
                sl = slice(sc * 512, (sc + 1) * 512)
                # k projection (duplicated into both halves)
                if sc == 0:
                    tensor.wait_ge(sL, 16)
                if sc >= 2:
                    tensor.wait_ge(sKC, sc - 1)
                for ht in range(NHT):
                    inst = tensor.matmul(
                        kp[sc % 2], wkk_sb[:, ht, :], hsT[:, ht, sl],
                        start=(ht == 0), stop=(ht == NHT - 1),
                    )
                tensor.drain().then_inc(sKP, 1)
                # q projection (2 heads packed)
                if sc == 0:
                    tensor.wait_ge(sL, 32)
                if sc >= 2:
                    tensor.wait_ge(sQC, sc - 1)
                for ht in range(NHT):
                    inst = tensor.matmul(
                        qp[sc % 2], wq_sb[:, ht, :], hsT[:, ht, sl],
                        start=(ht == 0), stop=(ht == NHT - 1),
                    )
                tensor.drain().then_inc(sQP, 1)
                # v projection
                if sc == 0:
                    tensor.wait_ge(sL, 48)
                if sc >= 2:
                    tensor.wait_ge(sVC, sc - 1)
                for ht in range(NHT):
                    inst = tensor.matmul(
                        vp[sc % 2], wv_sb[:, ht, :], hsT[:, ht, sl],
                        start=(ht == 0), stop=(ht == NHT - 1),
                    )
                tensor.drain().then_inc(sVP, 1)
                # v transposes for this chunk's 4 ktiles
                if sc == 0:
                    tensor.wait_ge(sL, 64)
                tensor.wait_ge(sVC, sc + 1)
                for j in range(4):
                    kt = 4 * sc + j
                    tensor.transpose(
                        vtp[:, kt * 64:(kt + 1) * 64],
                        vT[:, kt * 128:(kt + 1) * 128],
                        ident[0:HD, 0:HD],
                    )
                tensor.drain().then_inc(sVTD, 1)
            # ================= attention =================
            tensor.wait_ge(sQR, NSC)
            tensor.wait_ge(sKR, NSC)
            tensor.wait_ge(sVA, NST)
            tensor.wait_ge(sMS, 1)
            tensor.wait_ge(sQC, NSC)
            tensor.wait_ge(sKC, NSC)
            tensor.wait_ge(sVC, NSC)
            for h in range(2):
                hp = slice(h * 64, (h + 1) * 64)
                for kt in range(NST):
                    for qh in range(2):
                        u = h * 32 + kt * 2 + qh
                        if u >= 2:
                            tensor.wait_ge(sEX, u - 1)
                        for i in range(2):
                            tensor.matmul(
                                sp[u % 2][:, i * 512:(i + 1) * 512],
                                kT2[hp, kt * 128:(kt + 1) * 128],
                                qT2[hp, qh * 1024 + i * 512: qh * 1024 + (i + 1) * 512],
                                start=True, stop=True,
                            ).then_inc(sSC, 1)
                    if kt >= 1:
                        _pv(tensor, h, kt - 1)
                _pv(tensor, h, NST - 1)
                tensor.drain().then_inc(sPVD, 1)
            # ================= o_proj (two passes over slabs) ==========
            tensor.wait_ge(sEX, 64)
            tensor.wait_ge(sL, 112)
            # slab0 pass (af gating +1-shifted: next DMA's completion proves
            # this chunk's SBUF writes are visible)
            for oc in range(NSC):
                tensor.wait_ge(sAF, 16 * (oc + 2))
                for t in range(4):
                    tensor.matmul(
                        op4[oc], wo_sb[:, t, :],
                        af[0][:, t, oc * 512:(oc + 1) * 512],
                        start=(t == 0), stop=False, skip_group_check=True,
                    )
            # slab1 pass
            for oc in range(NSC):
                tensor.wait_ge(sAF, 16 * NSC + 16 * (oc + 2))
                for t in range(4):
                    tensor.matmul(
                        op4[oc], wo_sb[:, 4 + t, :],
                        af[1][:, t, oc * 512:(oc + 1) * 512],
                        start=False, stop=(t == 3), skip_group_check=True,
                    )
                tensor.drain().then_inc(sOP, 1)

        # ================= DVE =================
        @block.vector
        def _(vector):
            for sc in range(NSC):
                sl = slice(sc * 512, (sc + 1) * 512)
                vector.wait_ge(sQP, sc + 1)
                if sc >= 2:
                    vector.wait_ge(sSWQ, 64 * (sc - 1))  # q2 swap reads done
                vector.tensor_copy(q2[sc % 2], qp[sc % 2]).then_inc(sQC, 1)
                vector.wait_ge(sKP, sc + 1)
                if sc >= 2:
                    vector.wait_ge(sSWK, 64 * (sc - 1))  # k2 swap reads done
                vector.tensor_copy(k2[sc % 2], kp[sc % 2]).then_inc(sKC, 1)
                vector.wait_ge(sVP, sc + 1)
                vector.tensor_copy(vT[:, sl], vp[sc % 2]).then_inc(sVC, 1)
                # rope q (both heads packed)
                if sc == 0:
                    vector.wait_ge(sL, 96)
                vector.wait_ge(sSWQ, 64 * (sc + 1))
                vector.tensor_mul(tmpa, q2[sc % 2], cosT2[:, sl])
                vector.tensor_mul(tmpb, qs2[sc % 2], ssinT2[:, sl])
                vector.tensor_add(qT2[:, sl], tmpa, tmpb).then_inc(sQR, 1)
                # rope k
                vector.wait_ge(sSWK, 64 * (sc + 1))
                vector.tensor_mul(tmpa, k2[sc % 2], cosT2[:, sl])
                vector.tensor_mul(tmpb, ks2[sc % 2], ssinT2[:, sl])
                vector.tensor_add(kT2[:, sl], tmpa, tmpb).then_inc(sKR, 1)
                # vaug copies
                vector.wait_ge(sVTD, sc + 1)
                for j in range(4):
                    kt = 4 * sc + j
                    vector.tensor_copy(vaug[:, kt, 0:HD], vtp[:, kt * 64:(kt + 1) * 64]).then_inc(sVA, 1)
            # raw copies (release pv psum) + normalize
            for h in range(2):
                vector.wait_ge(sPVD, h + 1)
                for oc in range(NSC):
                    sl = slice(oc * 512, (oc + 1) * 512)
                    vector.tensor_copy(araw[h][:, sl], pv[:, sl]).then_inc(sRW, 1)
                vector.wait_ge(sDN64, 32 * (h + 1))
                if h == 1:
                    vector.wait_ge(sDNS, 16)  # rcp64 drained to scr2 (h0)
                vector.reciprocal(rcp64, den64).then_inc(sRC, 1)
                for oc in range(NSC):
                    sl = slice(oc * 512, (oc + 1) * 512)
                    # +1-DMA shift: sw-DGE completion sems can fire before the
                    # data is visible; the NEXT DMA's completion (same queue,
                    # FIFO) proves this one's writes landed
                    vector.wait_ge(sRB, 16 * (5 * h + oc + 2))
                    vector.tensor_mul(attn_sb[h][:, sl], araw[h][0:HD, sl], rb[oc]).then_inc(sNM, 1)
            # out copies
            for oc in range(NSC):
                vector.wait_ge(sOP, oc + 1)
                if oc >= 2:
                    vector.wait_ge(sOD, 16 * (oc - 1))
                vector.tensor_copy(out_ch[oc % 2], op4[oc]).then_inc(sOC, 1)

        # ====== ACT: hsT ch1/ch3 loads + swap DMAs + exp (hw DGE) ======
        @block.scalar
        def _(scalar):
            scalar.dma_start(out=hsT[:, :, 512:1024], in_=hsT_d[1]).then_inc(sHSb, 16)
            scalar.dma_start(out=hsT[:, :, 1536:2048], in_=hsT_d[3]).then_inc(sHSb, 16)
            for sc in range(NSC):
                scalar.wait_ge(sQC, sc + 1)
                if sc >= 2:
                    scalar.wait_ge(sQR, sc - 1)  # qs2 buf consumed
                for b in range(2):
                    scalar.dma_start(
                        out=qs2[sc % 2][b * 64:b * 64 + 32, :],
                        in_=q2[sc % 2][b * 64 + 32:b * 64 + 64, :],
                    ).then_inc(sSWQ, 16)
                    scalar.dma_start(
                        out=qs2[sc % 2][b * 64 + 32:b * 64 + 64, :],
                        in_=q2[sc % 2][b * 64:b * 64 + 32, :],
                    ).then_inc(sSWQ, 16)
                scalar.wait_ge(sKC, sc + 1)
                if sc >= 2:
                    scalar.wait_ge(sKR, sc - 1)  # ks2 buf consumed
                for b in range(2):
                    scalar.dma_start(
                        out=ks2[sc % 2][b * 64:b * 64 + 32, :],
                        in_=k2[sc % 2][b * 64 + 32:b * 64 + 64, :],
                    ).then_inc(sSWK, 16)
                    scalar.dma_start(
                        out=ks2[sc % 2][b * 64 + 32:b * 64 + 64, :],
                        in_=k2[sc % 2][b * 64:b * 64 + 32, :],
                    ).then_inc(sSWK, 16)
            # exps
            for h in range(2):
                for kt in range(NST):
                    for qh in range(2):
                        u = h * 32 + kt * 2 + qh
                        slot = (2 * kt + qh) % NPT
                        scalar.wait_ge(sSC, 2 * u + 2)
                        # PT[slot] was last written NPT//2 kt earlier (global
                        # kt order) and is consumed by that kt's PV group.
                        w = h * NST + kt - (NPT // 2 - 1)
                        if w >= 1:
                            scalar.wait_ge(sPV, w)
                        scalar.activation(
                            PT[slot][:, :], sp[u % 2], EXP, scale=0.125,
                        ).then_inc(sEX, 1)

        # ==== GPSIMD: memset, sw-DGE den/broadcast DMAs, collectives ====
        @block.gpsimd
        def _(gpsimd):
            gpsimd.memset(vaug[:, :, HD:HD + 1], 1.0).then_inc(sMS, 1)
            for h in range(2):
                gpsimd.wait_ge(sRW, 4 * (h + 1))
                gpsimd.dma_start(
                    out=den64, in_=araw[h][HD:HD + 1, :],
                ).then_inc(sDN64, 16)
                # dummy follow-up DMA: its completion proves den64 is visible
                # (sw-DGE completion sems can fire before the data lands)
                gpsimd.dma_start(
                    out=dum_sb, in_=araw[h][HD:HD + 1, 0:16],
                ).then_inc(sDN64, 16)
                gpsimd.wait_ge(sDNS, 32 * (h + 1))
                for oc in range(NSC):
                    gpsimd.dma_start(
                        out=rb[oc],
                        in_=bass.AP(scr2_d[:].tensor, h * S + oc * 512, [[0, HD], [1, 512]]),
                    ).then_inc(sRB, 16)
                gpsimd.dma_start(
                    out=dum_sb, in_=bass.AP(scr2_d[:].tensor, h * S, [[0, 1], [1, 16]]),
                ).then_inc(sRB, 16)
                gpsimd.wait_ge(sAG, 32 * (h + 1))
                gpsimd.collective_compute(
                    "AllGather",
                    mybir.AluOpType.bypass,
                    replica_groups=[list(range(NCORES))],
                    ins=[agin_d[h]],
                    outs=[agout_d[h]],
                ).then_inc(sCC, 1)
            # barrier collective: its completion proves every rank's cc1
            # pushes into our agout landed (FIFO per sender SDMA queue)
            gpsimd.collective_compute(
                "AllGather",
                mybir.AluOpType.bypass,
                replica_groups=[list(range(NCORES))],
                ins=[bar_d[:]],
                outs=[barout_d[:]],
            ).then_inc(sCC, 1)

    es.close()
    return nc


_NC_CACHE = None


def kernel(hidden_states, cos, sin, attention_mask, Wq, Wk, Wv, Wo):
    global _NC_CACHE
    if _NC_CACHE is None:
        _NC_CACHE = build_kernel()
    nc = _NC_CACHE
    hs2 = np.asarray(hidden_states, dtype=np.float32).reshape(S, HID)
    # hsT chunk-contiguous: [sc, p, t*512] with row (t*128+p) of hs.T
    hsT = np.ascontiguousarray(hs2.T.astype(NPBF))                    # [HID, S]
    hsT_c = np.ascontiguousarray(
        hsT.reshape(NHT, 128, NSC, 512).transpose(2, 1, 0, 3).reshape(NSC, 128, NHT * 512))
    cosT = np.asarray(cos, dtype=np.float32).reshape(S, HD).T         # [64, S]
    sinT = np.asarray(sin, dtype=np.float32).reshape(S, HD).T
    ssinT = sinT.copy()
    ssinT[0:32, :] *= -1.0
    cosT2 = np.ascontiguousarray(np.concatenate([cosT, cosT], 0).astype(NPBF))
    ssinT2 = np.ascontiguousarray(np.concatenate([ssinT, ssinT], 0).astype(NPBF))
    Wq = np.asarray(Wq, dtype=np.float32)
    Wk = np.asarray(Wk, dtype=np.float32)
    Wv = np.asarray(Wv, dtype=np.float32)
    Wo = np.asarray(Wo, dtype=np.float32)
    ident = np.eye(128, dtype=np.float32).astype(NPBF)

    def warr(w):  # [1024, X] -> [128, 8*X] partition-major contiguous
        x = w.shape[1]
        return np.ascontiguousarray(
            w.reshape(NHT, 128, x).transpose(1, 0, 2).reshape(128, NHT * x).astype(NPBF))

    # slab row order for Wo: row (j, p) = (2*(2t + a) + h)*64 + d,
    # j = h*4 + t, p = a*64 + d
    order = np.empty(HID, dtype=np.int64)
    for j in range(8):
        h, t = j // 4, j % 4
        for p in range(128):
            a, d = p // 64, p % 64
            order[j * 128 + p] = (2 * (2 * t + a) + h) * 64 + d
    in_maps = []
    for c in range(NCORES):
        g = c // 2
        wk_g = Wk[:, g * HD:(g + 1) * HD]
        in_maps.append({
            "hst": hsT_c,
            "wq": warr(Wq[:, c * 128:(c + 1) * 128]),
            "wkk": warr(np.concatenate([wk_g, wk_g], axis=1)),
            "wv": warr(Wv[:, g * HD:(g + 1) * HD]),
            "wo": np.ascontiguousarray(
                Wo[order, c * 128:(c + 1) * 128].astype(NPBF)
                .reshape(NHT, 128, 128).transpose(1, 0, 2).reshape(128, NHT * 128)),
            "cost": cosT2,
            "ssint": ssinT2,
            "ident": ident,
        })
    res = run_bass_kernel_spmd(nc, in_maps, core_ids=list(range(NCORES)),
                               trace=bool(int(os.environ.get("KERNEL_TRACE", "0"))))
    out = np.empty((S, HID), dtype=np.float32)
    for c in range(NCORES):
        out[:, c * 128:(c + 1) * 128] = res.results[c]["out_t"].T
    kernel.last_results = res
    return out.reshape(1, S, HID)


if __name__ == "__main__":
    import tempfile
    from concourse.bass_utils import compile_bass_kernel
    nc = build_kernel()
    with tempfile.TemporaryDirectory() as td:
        compile_bass_kernel(nc, td)
    print("COMPILE OK")
